# revision 53
# baseline (speedup 1.0000x reference)
"""Multi-head self-attention (B=8, T=2048, C=192, H=6, HS=32) on 8 TRN2 cores.

Sharding: data-parallel over batch - core i computes batch element i fully
on-chip (no collectives).

v2 design (cost model: matmul = out-free-cols * pe_cycle; ACT/DVE/Pool =
free-cols * engine cycle):
  - qT/kT [d, t] projections as in v1 (PSUM->SBUF copies on ACT).
  - v_aug [s, 6*33] = x @ Wv_aug with a built-in ones column per head
    (host pads xT with a ones row; Wv_aug carries the ones pattern).
  - S^T [s, t] per head: 1 matmul per (head, s-tile, t-chunk), K=32.
  - P^T = exp(S^T/sqrt(HS)) split across three engines:
      ACT:  exact activation-Exp
      DVE:  Schraudolph bf16-bits exp (int16(A*x+B) bitcast to bf16)
      Pool: same trick from an fp16 staging copy made by ACT/DVE
            (GPSIMD cannot read PSUM)
  - AV in [t, d] orientation: out [t-tile 128, 33] per (t-tile, head,
    s-block) accumulated over 16 s-blocks; col 32 = rowsum via the ones col.
  - normalize via DVE broadcast mul, ones col appended -> On [128, 193] f32
  - PE transpose (fp32, via identity) -> O^T in reused AV psum banks,
    ACT/DVE copy to SBUF -> otnT_a [97, 128] (row 96 = softmax ones ->
    bias via Wp_a row 96 = bproj), otnT_b [96, 128]
  - out projection: 2 matmuls (K=97/96) + DVE copy + DMA per t-tile.
"""

import numpy as np
import ml_dtypes
from contextlib import ExitStack

import concourse.bass as bass
import concourse.tile as tile
from concourse import bacc, mybir
from concourse.bass_utils import run_bass_kernel_spmd

B, T, C = 8, 2048, 192
H, HS = 6, 32
P = 128
TCH = 512            # t-chunk width of one S^T tile pair
NT = T // TCH        # 4
NS = T // P          # 16 s-tiles / t-tiles
SCALE = 1.0 / float(np.sqrt(HS))
BF16 = mybir.dt.bfloat16
F16 = mybir.dt.float16
F32 = mybir.dt.float32
I16 = mybir.dt.int16
Exp = mybir.ActivationFunctionType.Exp
Copy = mybir.ActivationFunctionType.Copy
MUL = mybir.AluOpType.mult
ADD = mybir.AluOpType.add

# Schraudolph constants for bf16-bits exp: bf16_bits = int16(A*x + B)
SCH_A = 128.0 / np.log(2.0)
SCH_B = 127.0 * 128.0 - 7.5 + 0.5

# exp-engine split (192 tiles of [128, 1024]): ACT direct / DVE direct.
# Pool exp is a net loss (GPSIMD can't read PSUM and the staging copy
# costs the copier engine as much as doing the exp directly).
N_ACT, N_DVE, N_POOL = 108, 84, 0
N_POOLCP_ACT = 0      # of the pool tiles, how many staging copies ACT makes
N_EXP = 192

_CACHE = {}


def _exp_plan():
    """Weighted round-robin: list of (engine, copier) for the exp tiles."""
    plan = []
    acc = {"act": 0.0, "dve": 0.0, "pool": 0.0}
    tot = float(N_EXP)
    w = {"act": N_ACT / tot, "dve": N_DVE / tot, "pool": N_POOL / tot}
    n = {"act": 0, "dve": 0, "pool": 0}
    cap = {"act": N_ACT, "dve": N_DVE, "pool": N_POOL}
    cp_acc, cp_n = 0.0, 0
    for _ in range(N_EXP):
        for k in acc:
            acc[k] += w[k]
        pick = max((k for k in acc if n[k] < cap[k]), key=lambda k: acc[k])
        acc[pick] -= 1.0
        n[pick] += 1
        copier = None
        if pick == "pool":
            cp_acc += N_POOLCP_ACT / float(N_POOL)
            if cp_acc >= 1.0 and cp_n < N_POOLCP_ACT:
                cp_acc -= 1.0
                cp_n += 1
                copier = "act"
            else:
                copier = "dve"
        plan.append((pick, copier))
    return plan


def build_nc():
    nc = bacc.Bacc()
    xT = nc.declare_dram_parameter("xT", [C + 1, T], BF16, isOutput=False)
    wq = nc.declare_dram_parameter("wq", [C, H * HS], BF16, isOutput=False)
    wk = nc.declare_dram_parameter("wk", [C, H * HS], BF16, isOutput=False)
    wv = nc.declare_dram_parameter("wv", [C + 1, H * 33], BF16, isOutput=False)
    wpa = nc.declare_dram_parameter("wpa", [96, C], BF16, isOutput=False)
    wpb = nc.declare_dram_parameter("wpb", [97, C], BF16, isOutput=False)
    idn = nc.declare_dram_parameter("idn", [P, P], BF16, isOutput=False)
    out = nc.declare_dram_parameter("out", [T, C], F32, isOutput=True)

    plan = _exp_plan()

    with tile.TileContext(nc) as tc, ExitStack() as ctx:
        singles = ctx.enter_context(tc.tile_pool(name="singles", bufs=1))
        qk_pool = ctx.enter_context(tc.tile_pool(name="qk", bufs=1))
        vaug_pool = ctx.enter_context(tc.tile_pool(name="vaug", bufs=1))
        pt_pool = ctx.enter_context(tc.tile_pool(name="ptp", bufs=18))
        on_pool = ctx.enter_context(tc.tile_pool(name="onp", bufs=4))
        ot_pool = ctx.enter_context(tc.tile_pool(name="otp", bufs=4))
        small = ctx.enter_context(tc.tile_pool(name="small", bufs=4))
        ysb_pool = ctx.enter_context(tc.tile_pool(name="ysb", bufs=4))

        # ---------------- load inputs ----------------
        # Spread the loads over four engine DMA queues so the first q/k
        # projection group's inputs (wq a/b + x chunk 0) are all in flight
        # immediately instead of serializing on the SP queue.
        w_sb = {}
        wqa_t = singles.tile([P, H * HS], BF16, name="wqa")
        nc.sync.dma_start(wqa_t, wq[0:P, :])
        wqb_t = singles.tile([C - P, H * HS], BF16, name="wqb")
        nc.scalar.dma_start(wqb_t, wq[P:C, :])
        wka_t = singles.tile([P, H * HS], BF16, name="wka")
        nc.gpsimd.dma_start(wka_t, wk[0:P, :])
        wkb_t = singles.tile([C - P, H * HS], BF16, name="wkb")
        nc.gpsimd.dma_start(wkb_t, wk[P:C, :])
        w_sb["q"] = (wqa_t, wqb_t)
        w_sb["k"] = (wka_t, wkb_t)

        xa = singles.tile([P, T], BF16)
        xb = singles.tile([C + 1 - P, T], BF16)          # 65 rows (ones last)
        nc.sync.dma_start(xa[:, 0:T // 2], xT[0:P, 0:T // 2])
        nc.scalar.dma_start(xb[:, 0:T // 2], xT[P:C + 1, 0:T // 2])
        nc.sync.dma_start(xa[:, T // 2:T], xT[0:P, T // 2:T])
        nc.sync.dma_start(xb[:, T // 2:T], xT[P:C + 1, T // 2:T])

        wva = singles.tile([P, H * 33], BF16)
        nc.gpsimd.dma_start(wva, wv[0:P, :])
        wvb = singles.tile([C + 1 - P, H * 33], BF16)
        nc.gpsimd.dma_start(wvb, wv[P:C + 1, :])
        wpa_sb = singles.tile([96, C], BF16)
        nc.gpsimd.dma_start(wpa_sb, wpa[:, :])
        wpb_sb = singles.tile([97, C], BF16)
        nc.gpsimd.dma_start(wpb_sb, wpb[:, :])
        idn_sb = singles.tile([P, P], BF16)
        nc.gpsimd.dma_start(idn_sb, idn[:, :])

        # PE p-state warmup: ~3.5us of dummy matmuls from a memset tile so
        # the ramp (full clock after 3us continuous busy) completes before
        # the real projections start. PE is idle during the loads anyway.
        warm = singles.tile([P, TCH], BF16)
        nc.gpsimd.memset(warm, 0.0)
        with tc.tile_pool(name="pwarm", bufs=1, space="PSUM") as pwarm:
            wps = pwarm.tile([P, TCH], F32)
            for _ in range(8):
                nc.tensor.matmul(wps, warm[:, 0:P], warm,
                                 start=True, stop=True)

        # ---------------- phase 1: qT, kT, v_aug ----------------
        qT_a = qk_pool.tile([P, T], BF16)       # heads 0..3, d-major
        qT_b = qk_pool.tile([C - P, T], BF16)   # heads 4,5
        kT_a = qk_pool.tile([P, T], BF16)
        kT_b = qk_pool.tile([C - P, T], BF16)
        v_aug = []
        with tc.tile_pool(name="pqk", bufs=2, space="PSUM") as pqk, \
             tc.tile_pool(name="pv", bufs=2, space="PSUM") as pv:
            def v_group(si):
                s0 = si * P
                ps = pv.tile([P, H * 33], F32, name="psv", tag="psv")
                nc.tensor.matmul(ps, xa[:, s0:s0 + P], wva,
                                 start=True, stop=False)
                nc.tensor.matmul(ps, xb[:, s0:s0 + P], wvb,
                                 start=False, stop=True)
                va = vaug_pool.tile([P, H * 33], BF16,
                                    name=f"vaug{si}", tag=f"vaug{si}")
                nc.vector.tensor_copy(va, ps)
                v_aug.append(va)

            # interleave q/k psum groups with v groups so ACT (psq copies)
            # and DVE (vaug copies) both get fed from the start; alternate
            # q/k and put low-t chunks first so QKT(tc0=0) can begin early
            qk_groups = []
            for t0 in range(0, T, 2 * TCH):
                for dlo, dsz, ia in ((0, P, 0), (P, C - P, 1)):
                    qk_groups.append(("q", dlo, dsz,
                                      (qT_a, qT_b)[ia], t0))
                    qk_groups.append(("k", dlo, dsz,
                                      (kT_a, kT_b)[ia], t0))
            vi = 0
            for gi, (proj, dlo, dsz, dst, t0) in enumerate(qk_groups):
                wa, wb = w_sb[proj]
                ps = pqk.tile([P, 2 * TCH], F32, name="psq", tag="psq")
                for tt0 in (t0, t0 + TCH):
                    c0 = tt0 - t0
                    nc.tensor.matmul(
                        ps[0:dsz, c0:c0 + TCH], wa[:, dlo:dlo + dsz],
                        xa[:, tt0:tt0 + TCH], start=True, stop=False)
                    nc.tensor.matmul(
                        ps[0:dsz, c0:c0 + TCH], wb[:, dlo:dlo + dsz],
                        xb[0:C - P, tt0:tt0 + TCH],
                        start=False, stop=True)
                nc.scalar.activation(
                    dst[0:dsz, t0:t0 + 2 * TCH], ps[0:dsz, :], Copy,
                    scale=1.0)
                while vi < NS and vi <= 2 * gi:
                    v_group(vi)
                    vi += 1
            while vi < NS:
                v_group(vi)
                vi += 1

        # ---------------- phase 2: attention ----------------
        def hsrc(h):
            if h < 4:
                return kT_a, qT_a, HS * h
            return kT_b, qT_b, HS * (h - 4)

        exp_i = [0]

        with (
            tc.tile_pool(name="pst", bufs=3, space="PSUM") as pst_pool,
            tc.tile_pool(name="pav", bufs=1, space="PSUM") as pav_pool,
        ):
            pending_tail = [None]
            pending_av = [None]

            def flush_tail():
                if pending_tail[0] is not None:
                    pending_tail[0]()
                    pending_tail[0] = None

            for tci, tc0 in enumerate(range(0, T, TCH)):
                # two av accumulators, each one PSUM bank: cols =
                # (tt%2)*198 + h*33 + [0..32]; col 32 of each head = rowsum
                av = [pav_pool.tile([P, 396], F32, name=f"av{b}", tag=f"av{b}")
                      for b in range(2)]
                pend = {}

                def issue_av(si, av=av, pend=pend):
                    for tt in range(4):
                        b, jj = tt // 2, tt % 2
                        for h in range(H):
                            g, j = h // 2, h % 2
                            ptp = pend[si][g]
                            nc.tensor.matmul(
                                av[b][:, jj * 198 + h * 33:
                                      jj * 198 + (h + 1) * 33],
                                ptp[:, j * TCH + tt * P:
                                    j * TCH + tt * P + P],
                                v_aug[si][:, h * 33:(h + 1) * 33],
                                # start=True marks the WHOLE psum bank
                                # pending-zero, so only the first chain into
                                # each bank may set it; the other chains'
                                # first writes then overwrite pending-zero
                                # bytes (= implicit zero init).
                                start=(si == 0 and jj == 0 and h == 0),
                                stop=(si == NS - 1),
                                skip_group_check=True)

                for si in range(NS):
                    s0 = si * P
                    if si == 2:
                        # emit the previous tc0's tail now: its norms must
                        # precede this tc0's first AV writes (WAR on the av
                        # banks), but deferring it past si=0/1 lets the PE
                        # keep the exp engines fed across the boundary.
                        flush_tail()
                    cur = []
                    for g in range(3):
                        stp = pst_pool.tile([P, 2 * TCH], F32,
                                            name="stp", tag="stp")
                        for j in range(2):
                            h = 2 * g + j
                            kT_t, qT_t, pb = hsrc(h)
                            nc.tensor.matmul(
                                stp[:, j * TCH:(j + 1) * TCH],
                                kT_t[pb:pb + HS, s0:s0 + P],
                                qT_t[pb:pb + HS, tc0:tc0 + TCH],
                                start=True, stop=True, tile_position=(pb, 0))
                        eng, copier = plan[exp_i[0]]
                        exp_i[0] += 1
                        ptp = pt_pool.tile([P, 2 * TCH], BF16,
                                           name="ptp", tag="ptp")
                        if eng == "act":
                            nc.scalar.activation(ptp, stp, Exp, scale=SCALE)
                        else:
                            nc.vector.tensor_scalar(
                                ptp.bitcast(I16), stp, SCH_A * SCALE, SCH_B,
                                op0=MUL, op1=ADD)
                        cur.append(ptp)
                    pend[si] = cur
                    # AV for si-2 issued AFTER this si's QKT groups: the PE
                    # produces stp tiles (exp-engine food) first each round.
                    if si >= 2:
                        issue_av(si - 2)
                issue_av(NS - 2)
                issue_av(NS - 1)

                def tail(av=av, tc0=tc0):
                    # normalize all 4 t-tiles first (frees av banks)
                    # On layout: [h0..h5 (cols 0:192) | ones (col 192)];
                    # group a = cols 0:96, group b = cols 96:193 so the
                    # ones col transposes into otb row 96 (bias row of wpb)
                    ons = []
                    for tt in range(4):
                        b, off = tt // 2, (tt % 2) * 198
                        avr = av[b][:, off:off + 198].rearrange(
                            "p (h e) -> p h e", h=H)
                        rrec = small.tile([P, H], F32, name="rrec", tag="rrec")
                        nc.vector.reciprocal(rrec[:, :, None],
                                             avr[:, :, 32:33])
                        on = on_pool.tile([P, 193], BF16, name="on", tag="on")
                        nc.vector.tensor_tensor(
                            on[:, 0:192].rearrange("p (h e) -> p h e", h=H),
                            avr[:, :, 0:32],
                            rrec[:, :, None].to_broadcast((P, H, 32)),
                            op=MUL)
                        nc.gpsimd.memset(on[:, 192:193], 1.0)
                        ons.append(on)

                    # transpose + project, reusing the freed av banks:
                    # av[0] (bf16 view) holds three 128-col transpose slots,
                    # av[1] holds two 192-col projection-psum slots.
                    av0b = av[0].bitcast(BF16)
                    for tt in range(4):
                        on = ons[tt]
                        ca = 128 * ((2 * tt) % 3)
                        cb = 128 * ((2 * tt + 1) % 3)
                        ga = av0b[:, ca:ca + 128]
                        gb = av0b[:, cb:cb + 128]
                        nc.tensor.transpose(ga[0:96, :], on[:, 0:96], idn_sb)
                        ota = ot_pool.tile([96, P], BF16, name="ota",
                                           tag="ota")
                        nc.vector.tensor_copy(ota, ga[0:96, :])
                        nc.tensor.transpose(gb[0:97, :], on[:, 96:193],
                                            idn_sb)
                        otb = ot_pool.tile([97, P], BF16, name="otb",
                                           tag="otb")
                        nc.vector.tensor_copy(otb, gb[0:97, :])
                        py = av[1][:, (tt % 2) * 192:(tt % 2) * 192 + 192]
                        nc.tensor.matmul(py, ota, wpa_sb,
                                         start=True, stop=False,
                                         skip_group_check=True)
                        nc.tensor.matmul(py, otb, wpb_sb,
                                         start=False, stop=True,
                                         skip_group_check=True)
                        ysb = ysb_pool.tile([P, C], F32, name="ysbt",
                                            tag="ysbt")
                        nc.vector.tensor_copy(ysb, py)
                        nc.sync.dma_start(
                            out[tc0 + tt * P:tc0 + (tt + 1) * P, :], ysb)

                pending_tail[0] = tail
            flush_tail()

    nc.compile()
    return nc


def _get_nc():
    if "nc" not in _CACHE:
        _CACHE["nc"] = build_nc()
    return _CACHE["nc"]


def make_in_maps(x, Wq, Wk, Wv, Wproj, bproj):
    bf = ml_dtypes.bfloat16
    x = np.asarray(x, np.float32)
    pack = lambda w: np.ascontiguousarray(
        np.transpose(np.asarray(w, np.float32), (1, 0, 2)).reshape(C, H * HS)
    ).astype(bf)
    wq, wk = pack(Wq), pack(Wk)

    wv_aug = np.zeros((C + 1, H * 33), np.float32)
    Wv = np.asarray(Wv, np.float32)
    for h in range(H):
        wv_aug[0:C, h * 33:h * 33 + 32] = Wv[h]
        wv_aug[C, h * 33 + 32] = 1.0
    wv_aug = wv_aug.astype(bf)

    Wp = np.asarray(Wproj, np.float32)          # [H*HS, C]
    wpa = np.ascontiguousarray(Wp[0:96]).astype(bf)
    wpb = np.zeros((97, C), np.float32)
    wpb[0:96] = Wp[96:192]
    wpb[96] = np.asarray(bproj, np.float32)
    wpb = wpb.astype(bf)

    idn = np.eye(P, dtype=bf)

    maps = []
    for i in range(B):
        xp = np.ones((C + 1, T), np.float32)
        xp[0:C] = x[i].T
        maps.append({"xT": xp.astype(bf), "wq": wq, "wk": wk,
                     "wv": wv_aug, "wpa": wpa, "wpb": wpb, "idn": idn})
    return maps


def run(inputs, trace=False, **kw):
    nc = _get_nc()
    in_maps = make_in_maps(**inputs)
    res = run_bass_kernel_spmd(nc, in_maps, core_ids=list(range(B)),
                               trace=trace, **kw)
    y = np.stack([np.asarray(res.results[i]["out"], np.float32)
                  for i in range(B)], axis=0)
    return y, res


def kernel(**inputs):
    y, _ = run(inputs, trace=False)
    return y


# revision 54
# speedup vs baseline: 1.0347x; 1.0347x over previous
"""Multi-head self-attention (B=8, T=2048, C=192, H=6, HS=32) on 8 TRN2 cores.

Sharding: data-parallel over batch - core i computes batch element i fully
on-chip (no collectives).

v2 design (cost model: matmul = out-free-cols * pe_cycle; ACT/DVE/Pool =
free-cols * engine cycle):
  - qT/kT [d, t] projections as in v1 (PSUM->SBUF copies on ACT).
  - v_aug [s, 6*33] = x @ Wv_aug with a built-in ones column per head
    (host pads xT with a ones row; Wv_aug carries the ones pattern).
  - S^T [s, t] per head: 1 matmul per (head, s-tile, t-chunk), K=32.
  - P^T = exp(S^T/sqrt(HS)) split across three engines:
      ACT:  exact activation-Exp
      DVE:  Schraudolph bf16-bits exp (int16(A*x+B) bitcast to bf16)
      Pool: same trick from an fp16 staging copy made by ACT/DVE
            (GPSIMD cannot read PSUM)
  - AV in [t, d] orientation: out [t-tile 128, 33] per (t-tile, head,
    s-block) accumulated over 16 s-blocks; col 32 = rowsum via the ones col.
  - normalize via DVE broadcast mul, ones col appended -> On [128, 193] f32
  - PE transpose (fp32, via identity) -> O^T in reused AV psum banks,
    ACT/DVE copy to SBUF -> otnT_a [97, 128] (row 96 = softmax ones ->
    bias via Wp_a row 96 = bproj), otnT_b [96, 128]
  - out projection: 2 matmuls (K=97/96) + DVE copy + DMA per t-tile.
"""

import numpy as np
import ml_dtypes
from contextlib import ExitStack

import concourse.bass as bass
import concourse.tile as tile
from concourse import bacc, mybir
from concourse.bass_utils import run_bass_kernel_spmd

B, T, C = 8, 2048, 192
H, HS = 6, 32
P = 128
TCH = 512            # t-chunk width of one S^T tile pair
NT = T // TCH        # 4
NS = T // P          # 16 s-tiles / t-tiles
SCALE = 1.0 / float(np.sqrt(HS))
BF16 = mybir.dt.bfloat16
F16 = mybir.dt.float16
F32 = mybir.dt.float32
I16 = mybir.dt.int16
Exp = mybir.ActivationFunctionType.Exp
Copy = mybir.ActivationFunctionType.Copy
MUL = mybir.AluOpType.mult
ADD = mybir.AluOpType.add

# Schraudolph constants for bf16-bits exp: bf16_bits = int16(A*x + B)
SCH_A = 128.0 / np.log(2.0)
SCH_B = 127.0 * 128.0 - 7.5 + 0.5

# exp-engine split (192 tiles of [128, 1024]): ACT direct / DVE direct.
# Pool exp is a net loss (GPSIMD can't read PSUM and the staging copy
# costs the copier engine as much as doing the exp directly).
N_ACT, N_DVE, N_POOL = 101, 91, 0
N_POOLCP_ACT = 0      # of the pool tiles, how many staging copies ACT makes
N_EXP = 192

_CACHE = {}


def _exp_plan():
    """Weighted round-robin: list of (engine, copier) for the exp tiles."""
    plan = []
    acc = {"act": 0.0, "dve": 0.0, "pool": 0.0}
    tot = float(N_EXP)
    w = {"act": N_ACT / tot, "dve": N_DVE / tot, "pool": N_POOL / tot}
    n = {"act": 0, "dve": 0, "pool": 0}
    cap = {"act": N_ACT, "dve": N_DVE, "pool": N_POOL}
    cp_acc, cp_n = 0.0, 0
    for _ in range(N_EXP):
        for k in acc:
            acc[k] += w[k]
        pick = max((k for k in acc if n[k] < cap[k]), key=lambda k: acc[k])
        acc[pick] -= 1.0
        n[pick] += 1
        copier = None
        if pick == "pool":
            cp_acc += N_POOLCP_ACT / float(N_POOL)
            if cp_acc >= 1.0 and cp_n < N_POOLCP_ACT:
                cp_acc -= 1.0
                cp_n += 1
                copier = "act"
            else:
                copier = "dve"
        plan.append((pick, copier))
    return plan


def build_nc():
    nc = bacc.Bacc()
    xT = nc.declare_dram_parameter("xT", [C + 1, T], BF16, isOutput=False)
    wq = nc.declare_dram_parameter("wq", [C, H * HS], BF16, isOutput=False)
    wk = nc.declare_dram_parameter("wk", [C, H * HS], BF16, isOutput=False)
    wv = nc.declare_dram_parameter("wv", [C + 1, H * 33], BF16, isOutput=False)
    wpa = nc.declare_dram_parameter("wpa", [96, C], BF16, isOutput=False)
    wpb = nc.declare_dram_parameter("wpb", [97, C], BF16, isOutput=False)
    idn = nc.declare_dram_parameter("idn", [P, P], BF16, isOutput=False)
    out = nc.declare_dram_parameter("out", [T, C], F32, isOutput=True)

    plan = _exp_plan()

    with tile.TileContext(nc) as tc, ExitStack() as ctx:
        singles = ctx.enter_context(tc.tile_pool(name="singles", bufs=1))
        qk_pool = ctx.enter_context(tc.tile_pool(name="qk", bufs=1))
        vaug_pool = ctx.enter_context(tc.tile_pool(name="vaug", bufs=1))
        pt_pool = ctx.enter_context(tc.tile_pool(name="ptp", bufs=18))
        on_pool = ctx.enter_context(tc.tile_pool(name="onp", bufs=4))
        ot_pool = ctx.enter_context(tc.tile_pool(name="otp", bufs=2))
        small = ctx.enter_context(tc.tile_pool(name="small", bufs=4))
        ysb_pool = ctx.enter_context(tc.tile_pool(name="ysb", bufs=3))

        # ---------------- load inputs ----------------
        # Spread the loads over four engine DMA queues so the first q/k
        # projection group's inputs (wq a/b + x chunk 0) are all in flight
        # immediately instead of serializing on the SP queue.
        w_sb = {}
        wqa_t = singles.tile([P, H * HS], BF16, name="wqa")
        nc.sync.dma_start(wqa_t, wq[0:P, :])
        wqb_t = singles.tile([C - P, H * HS], BF16, name="wqb")
        nc.scalar.dma_start(wqb_t, wq[P:C, :])
        wka_t = singles.tile([P, H * HS], BF16, name="wka")
        nc.gpsimd.dma_start(wka_t, wk[0:P, :])
        wkb_t = singles.tile([C - P, H * HS], BF16, name="wkb")
        nc.gpsimd.dma_start(wkb_t, wk[P:C, :])
        w_sb["q"] = (wqa_t, wqb_t)
        w_sb["k"] = (wka_t, wkb_t)

        xa = singles.tile([P, T], BF16)
        xb = singles.tile([C + 1 - P, T], BF16)          # 65 rows (ones last)
        nc.sync.dma_start(xa[:, 0:T // 2], xT[0:P, 0:T // 2])
        nc.scalar.dma_start(xb[:, 0:T // 2], xT[P:C + 1, 0:T // 2])
        nc.sync.dma_start(xa[:, T // 2:T], xT[0:P, T // 2:T])
        nc.sync.dma_start(xb[:, T // 2:T], xT[P:C + 1, T // 2:T])

        wva = singles.tile([P, H * 33], BF16)
        nc.gpsimd.dma_start(wva, wv[0:P, :])
        wvb = singles.tile([C + 1 - P, H * 33], BF16)
        nc.gpsimd.dma_start(wvb, wv[P:C + 1, :])
        wpa_sb = singles.tile([96, C], BF16)
        nc.gpsimd.dma_start(wpa_sb, wpa[:, :])
        wpb_sb = singles.tile([97, C], BF16)
        nc.gpsimd.dma_start(wpb_sb, wpb[:, :])
        idn_sb = singles.tile([P, P], BF16)
        nc.gpsimd.dma_start(idn_sb, idn[:, :])

        # PE p-state warmup: ~3.5us of dummy matmuls from a memset tile so
        # the ramp (full clock after 3us continuous busy) completes before
        # the real projections start. PE is idle during the loads anyway.
        warm = singles.tile([P, TCH], BF16)
        nc.gpsimd.memset(warm, 0.0)
        with tc.tile_pool(name="pwarm", bufs=1, space="PSUM") as pwarm:
            wps = pwarm.tile([P, TCH], F32)
            for _ in range(8):
                nc.tensor.matmul(wps, warm[:, 0:P], warm,
                                 start=True, stop=True)

        # ---------------- phase 1: qT, kT, v_aug ----------------
        qT_a = qk_pool.tile([P, T], BF16)       # heads 0..3, d-major
        qT_b = qk_pool.tile([C - P, T], BF16)   # heads 4,5
        kT_a = qk_pool.tile([P, T], BF16)
        kT_b = qk_pool.tile([C - P, T], BF16)
        v_aug = []
        with tc.tile_pool(name="pqk", bufs=2, space="PSUM") as pqk, \
             tc.tile_pool(name="pv", bufs=2, space="PSUM") as pv:
            def v_group(si):
                s0 = si * P
                ps = pv.tile([P, H * 33], F32, name="psv", tag="psv")
                nc.tensor.matmul(ps, xa[:, s0:s0 + P], wva,
                                 start=True, stop=False)
                nc.tensor.matmul(ps, xb[:, s0:s0 + P], wvb,
                                 start=False, stop=True)
                va = vaug_pool.tile([P, H * 33], BF16,
                                    name=f"vaug{si}", tag=f"vaug{si}")
                nc.vector.tensor_copy(va, ps)
                v_aug.append(va)

            # interleave q/k psum groups with v groups so ACT (psq copies)
            # and DVE (vaug copies) both get fed from the start; alternate
            # q/k and put low-t chunks first so QKT(tc0=0) can begin early
            qk_groups = []
            for t0 in range(0, T, 2 * TCH):
                for dlo, dsz, ia in ((0, P, 0), (P, C - P, 1)):
                    qk_groups.append(("q", dlo, dsz,
                                      (qT_a, qT_b)[ia], t0))
                    qk_groups.append(("k", dlo, dsz,
                                      (kT_a, kT_b)[ia], t0))
            vi = 0
            for gi, (proj, dlo, dsz, dst, t0) in enumerate(qk_groups):
                wa, wb = w_sb[proj]
                ps = pqk.tile([P, 2 * TCH], F32, name="psq", tag="psq")
                for tt0 in (t0, t0 + TCH):
                    c0 = tt0 - t0
                    nc.tensor.matmul(
                        ps[0:dsz, c0:c0 + TCH], wa[:, dlo:dlo + dsz],
                        xa[:, tt0:tt0 + TCH], start=True, stop=False)
                    nc.tensor.matmul(
                        ps[0:dsz, c0:c0 + TCH], wb[:, dlo:dlo + dsz],
                        xb[0:C - P, tt0:tt0 + TCH],
                        start=False, stop=True)
                nc.scalar.activation(
                    dst[0:dsz, t0:t0 + 2 * TCH], ps[0:dsz, :], Copy,
                    scale=1.0)
                while vi < NS and vi <= 2 * gi:
                    v_group(vi)
                    vi += 1
            while vi < NS:
                v_group(vi)
                vi += 1

        # ---------------- phase 2: attention ----------------
        def hsrc(h):
            if h < 4:
                return kT_a, qT_a, HS * h
            return kT_b, qT_b, HS * (h - 4)

        exp_i = [0]

        with (
            tc.tile_pool(name="pst", bufs=3, space="PSUM") as pst_pool,
            tc.tile_pool(name="pav", bufs=1, space="PSUM") as pav_pool,
        ):
            pending_tail = [None]
            pending_av = [None]

            def flush_tail():
                if pending_tail[0] is not None:
                    pending_tail[0]()
                    pending_tail[0] = None

            for tci, tc0 in enumerate(range(0, T, TCH)):
                # two av accumulators, each one PSUM bank: cols =
                # (tt%2)*198 + h*33 + [0..32]; col 32 of each head = rowsum
                av = [pav_pool.tile([P, 396], F32, name=f"av{b}", tag=f"av{b}")
                      for b in range(2)]
                pend = {}

                def issue_av(si, av=av, pend=pend):
                    for tt in range(4):
                        b, jj = tt // 2, tt % 2
                        for h in range(H):
                            g, j = h // 2, h % 2
                            ptp = pend[si][g]
                            nc.tensor.matmul(
                                av[b][:, jj * 198 + h * 33:
                                      jj * 198 + (h + 1) * 33],
                                ptp[:, j * TCH + tt * P:
                                    j * TCH + tt * P + P],
                                v_aug[si][:, h * 33:(h + 1) * 33],
                                # start=True marks the WHOLE psum bank
                                # pending-zero, so only the first chain into
                                # each bank may set it; the other chains'
                                # first writes then overwrite pending-zero
                                # bytes (= implicit zero init).
                                start=(si == 0 and jj == 0 and h == 0),
                                stop=(si == NS - 1),
                                skip_group_check=True)

                for si in range(NS):
                    s0 = si * P
                    if si == 2:
                        # emit the previous tc0's tail now: its norms must
                        # precede this tc0's first AV writes (WAR on the av
                        # banks), but deferring it past si=0/1 lets the PE
                        # keep the exp engines fed across the boundary.
                        flush_tail()
                    cur = []
                    for g in range(3):
                        stp = pst_pool.tile([P, 2 * TCH], F32,
                                            name="stp", tag="stp")
                        for j in range(2):
                            h = 2 * g + j
                            kT_t, qT_t, pb = hsrc(h)
                            nc.tensor.matmul(
                                stp[:, j * TCH:(j + 1) * TCH],
                                kT_t[pb:pb + HS, s0:s0 + P],
                                qT_t[pb:pb + HS, tc0:tc0 + TCH],
                                start=True, stop=True, tile_position=(pb, 0))
                        eng, copier = plan[exp_i[0]]
                        exp_i[0] += 1
                        ptp = pt_pool.tile([P, 2 * TCH], BF16,
                                           name="ptp", tag="ptp")
                        if eng == "act":
                            nc.scalar.activation(ptp, stp, Exp, scale=SCALE)
                        else:
                            nc.vector.tensor_scalar(
                                ptp.bitcast(I16), stp, SCH_A * SCALE, SCH_B,
                                op0=MUL, op1=ADD)
                        cur.append(ptp)
                    pend[si] = cur
                    # AV for si-2 issued AFTER this si's QKT groups: the PE
                    # produces stp tiles (exp-engine food) first each round.
                    if si >= 2:
                        issue_av(si - 2)
                issue_av(NS - 2)
                issue_av(NS - 1)

                def tail(av=av, tc0=tc0):
                    # normalize all 4 t-tiles first (frees av banks)
                    # On layout: [h0..h5 (cols 0:192) | ones (col 192)];
                    # group a = cols 0:96, group b = cols 96:193 so the
                    # ones col transposes into otb row 96 (bias row of wpb)
                    ons = []
                    for tt in range(4):
                        b, off = tt // 2, (tt % 2) * 198
                        avr = av[b][:, off:off + 198].rearrange(
                            "p (h e) -> p h e", h=H)
                        rrec = small.tile([P, H], F32, name="rrec", tag="rrec")
                        nc.vector.reciprocal(rrec[:, :, None],
                                             avr[:, :, 32:33])
                        on = on_pool.tile([P, 193], BF16, name="on", tag="on")
                        nc.vector.tensor_tensor(
                            on[:, 0:192].rearrange("p (h e) -> p h e", h=H),
                            avr[:, :, 0:32],
                            rrec[:, :, None].to_broadcast((P, H, 32)),
                            op=MUL)
                        nc.gpsimd.memset(on[:, 192:193], 1.0)
                        ons.append(on)

                    # transpose + project, reusing the freed av banks:
                    # av[0] (bf16 view) holds three 128-col transpose slots,
                    # av[1] holds two 192-col projection-psum slots.
                    av0b = av[0].bitcast(BF16)
                    for tt in range(4):
                        on = ons[tt]
                        ca = 128 * ((2 * tt) % 3)
                        cb = 128 * ((2 * tt + 1) % 3)
                        ga = av0b[:, ca:ca + 128]
                        gb = av0b[:, cb:cb + 128]
                        nc.tensor.transpose(ga[0:96, :], on[:, 0:96], idn_sb)
                        ota = ot_pool.tile([96, P], BF16, name="ota",
                                           tag="ota")
                        nc.scalar.activation(ota, ga[0:96, :], Copy,
                                             scale=1.0)
                        nc.tensor.transpose(gb[0:97, :], on[:, 96:193],
                                            idn_sb)
                        otb = ot_pool.tile([97, P], BF16, name="otb",
                                           tag="otb")
                        nc.scalar.activation(otb, gb[0:97, :], Copy,
                                             scale=1.0)
                        py = av[1][:, (tt % 2) * 192:(tt % 2) * 192 + 192]
                        nc.tensor.matmul(py, ota, wpa_sb,
                                         start=True, stop=False,
                                         skip_group_check=True)
                        nc.tensor.matmul(py, otb, wpb_sb,
                                         start=False, stop=True,
                                         skip_group_check=True)
                        ysb = ysb_pool.tile([P, C], F32, name="ysbt",
                                            tag="ysbt")
                        nc.vector.tensor_copy(ysb, py)
                        nc.sync.dma_start(
                            out[tc0 + tt * P:tc0 + (tt + 1) * P, :], ysb)

                pending_tail[0] = tail
            flush_tail()

    nc.compile()
    return nc


def _get_nc():
    if "nc" not in _CACHE:
        _CACHE["nc"] = build_nc()
    return _CACHE["nc"]


def make_in_maps(x, Wq, Wk, Wv, Wproj, bproj):
    bf = ml_dtypes.bfloat16
    x = np.asarray(x, np.float32)
    pack = lambda w: np.ascontiguousarray(
        np.transpose(np.asarray(w, np.float32), (1, 0, 2)).reshape(C, H * HS)
    ).astype(bf)
    wq, wk = pack(Wq), pack(Wk)

    wv_aug = np.zeros((C + 1, H * 33), np.float32)
    Wv = np.asarray(Wv, np.float32)
    for h in range(H):
        wv_aug[0:C, h * 33:h * 33 + 32] = Wv[h]
        wv_aug[C, h * 33 + 32] = 1.0
    wv_aug = wv_aug.astype(bf)

    Wp = np.asarray(Wproj, np.float32)          # [H*HS, C]
    wpa = np.ascontiguousarray(Wp[0:96]).astype(bf)
    wpb = np.zeros((97, C), np.float32)
    wpb[0:96] = Wp[96:192]
    wpb[96] = np.asarray(bproj, np.float32)
    wpb = wpb.astype(bf)

    idn = np.eye(P, dtype=bf)

    maps = []
    for i in range(B):
        xp = np.ones((C + 1, T), np.float32)
        xp[0:C] = x[i].T
        maps.append({"xT": xp.astype(bf), "wq": wq, "wk": wk,
                     "wv": wv_aug, "wpa": wpa, "wpb": wpb, "idn": idn})
    return maps


def run(inputs, trace=False, **kw):
    nc = _get_nc()
    in_maps = make_in_maps(**inputs)
    res = run_bass_kernel_spmd(nc, in_maps, core_ids=list(range(B)),
                               trace=trace, **kw)
    y = np.stack([np.asarray(res.results[i]["out"], np.float32)
                  for i in range(B)], axis=0)
    return y, res


def kernel(**inputs):
    y, _ = run(inputs, trace=False)
    return y


# revision 55
# speedup vs baseline: 1.0378x; 1.0030x over previous
"""Multi-head self-attention (B=8, T=2048, C=192, H=6, HS=32) on 8 TRN2 cores.

Sharding: data-parallel over batch - core i computes batch element i fully
on-chip (no collectives).

v2 design (cost model: matmul = out-free-cols * pe_cycle; ACT/DVE/Pool =
free-cols * engine cycle):
  - qT/kT [d, t] projections as in v1 (PSUM->SBUF copies on ACT).
  - v_aug [s, 6*33] = x @ Wv_aug with a built-in ones column per head
    (host pads xT with a ones row; Wv_aug carries the ones pattern).
  - S^T [s, t] per head: 1 matmul per (head, s-tile, t-chunk), K=32.
  - P^T = exp(S^T/sqrt(HS)) split across three engines:
      ACT:  exact activation-Exp
      DVE:  Schraudolph bf16-bits exp (int16(A*x+B) bitcast to bf16)
      Pool: same trick from an fp16 staging copy made by ACT/DVE
            (GPSIMD cannot read PSUM)
  - AV in [t, d] orientation: out [t-tile 128, 33] per (t-tile, head,
    s-block) accumulated over 16 s-blocks; col 32 = rowsum via the ones col.
  - normalize via DVE broadcast mul, ones col appended -> On [128, 193] f32
  - PE transpose (fp32, via identity) -> O^T in reused AV psum banks,
    ACT/DVE copy to SBUF -> otnT_a [97, 128] (row 96 = softmax ones ->
    bias via Wp_a row 96 = bproj), otnT_b [96, 128]
  - out projection: 2 matmuls (K=97/96) + DVE copy + DMA per t-tile.
"""

import numpy as np
import ml_dtypes
from contextlib import ExitStack

import concourse.bass as bass
import concourse.tile as tile
from concourse import bacc, mybir
from concourse.bass_utils import run_bass_kernel_spmd

B, T, C = 8, 2048, 192
H, HS = 6, 32
P = 128
TCH = 512            # t-chunk width of one S^T tile pair
NT = T // TCH        # 4
NS = T // P          # 16 s-tiles / t-tiles
SCALE = 1.0 / float(np.sqrt(HS))
BF16 = mybir.dt.bfloat16
F16 = mybir.dt.float16
F32 = mybir.dt.float32
I16 = mybir.dt.int16
Exp = mybir.ActivationFunctionType.Exp
Copy = mybir.ActivationFunctionType.Copy
MUL = mybir.AluOpType.mult
ADD = mybir.AluOpType.add

# Schraudolph constants for bf16-bits exp: bf16_bits = int16(A*x + B)
SCH_A = 128.0 / np.log(2.0)
SCH_B = 127.0 * 128.0 - 7.5 + 0.5

# exp-engine split (192 tiles of [128, 1024]): ACT direct / DVE direct.
# Pool exp is a net loss (GPSIMD can't read PSUM and the staging copy
# costs the copier engine as much as doing the exp directly).
N_ACT, N_DVE, N_POOL = 105, 87, 0
N_POOLCP_ACT = 0      # of the pool tiles, how many staging copies ACT makes
N_EXP = 192

_CACHE = {}


def _exp_plan():
    """Weighted round-robin: list of (engine, copier) for the exp tiles."""
    plan = []
    acc = {"act": 0.0, "dve": 0.0, "pool": 0.0}
    tot = float(N_EXP)
    w = {"act": N_ACT / tot, "dve": N_DVE / tot, "pool": N_POOL / tot}
    n = {"act": 0, "dve": 0, "pool": 0}
    cap = {"act": N_ACT, "dve": N_DVE, "pool": N_POOL}
    cp_acc, cp_n = 0.0, 0
    for _ in range(N_EXP):
        for k in acc:
            acc[k] += w[k]
        pick = max((k for k in acc if n[k] < cap[k]), key=lambda k: acc[k])
        acc[pick] -= 1.0
        n[pick] += 1
        copier = None
        if pick == "pool":
            cp_acc += N_POOLCP_ACT / float(N_POOL)
            if cp_acc >= 1.0 and cp_n < N_POOLCP_ACT:
                cp_acc -= 1.0
                cp_n += 1
                copier = "act"
            else:
                copier = "dve"
        plan.append((pick, copier))
    return plan


def build_nc():
    nc = bacc.Bacc()
    xT = nc.declare_dram_parameter("xT", [C + 1, T], BF16, isOutput=False)
    wq = nc.declare_dram_parameter("wq", [C, H * HS], BF16, isOutput=False)
    wk = nc.declare_dram_parameter("wk", [C, H * HS], BF16, isOutput=False)
    wv = nc.declare_dram_parameter("wv", [C + 1, H * 33], BF16, isOutput=False)
    wpa = nc.declare_dram_parameter("wpa", [96, C], BF16, isOutput=False)
    wpb = nc.declare_dram_parameter("wpb", [97, C], BF16, isOutput=False)
    idn = nc.declare_dram_parameter("idn", [P, P], BF16, isOutput=False)
    out = nc.declare_dram_parameter("out", [T, C], F32, isOutput=True)

    plan = _exp_plan()

    with tile.TileContext(nc) as tc, ExitStack() as ctx:
        singles = ctx.enter_context(tc.tile_pool(name="singles", bufs=1))
        qk_pool = ctx.enter_context(tc.tile_pool(name="qk", bufs=1))
        vaug_pool = ctx.enter_context(tc.tile_pool(name="vaug", bufs=1))
        pt_pool = ctx.enter_context(tc.tile_pool(name="ptp", bufs=18))
        on_pool = ctx.enter_context(tc.tile_pool(name="onp", bufs=4))
        ot_pool = ctx.enter_context(tc.tile_pool(name="otp", bufs=2))
        small = ctx.enter_context(tc.tile_pool(name="small", bufs=4))
        ysb_pool = ctx.enter_context(tc.tile_pool(name="ysb", bufs=3))

        # ---------------- load inputs ----------------
        # Spread the loads over four engine DMA queues so the first q/k
        # projection group's inputs (wq a/b + x chunk 0) are all in flight
        # immediately instead of serializing on the SP queue.
        w_sb = {}
        wqa_t = singles.tile([P, H * HS], BF16, name="wqa")
        nc.sync.dma_start(wqa_t, wq[0:P, :])
        wqb_t = singles.tile([C - P, H * HS], BF16, name="wqb")
        nc.scalar.dma_start(wqb_t, wq[P:C, :])
        wka_t = singles.tile([P, H * HS], BF16, name="wka")
        nc.gpsimd.dma_start(wka_t, wk[0:P, :])
        wkb_t = singles.tile([C - P, H * HS], BF16, name="wkb")
        nc.gpsimd.dma_start(wkb_t, wk[P:C, :])
        w_sb["q"] = (wqa_t, wqb_t)
        w_sb["k"] = (wka_t, wkb_t)

        xa = singles.tile([P, T], BF16)
        xb = singles.tile([C + 1 - P, T], BF16)          # 65 rows (ones last)
        nc.sync.dma_start(xa[:, 0:T // 2], xT[0:P, 0:T // 2])
        nc.scalar.dma_start(xb[:, 0:T // 2], xT[P:C + 1, 0:T // 2])
        nc.sync.dma_start(xa[:, T // 2:T], xT[0:P, T // 2:T])
        nc.sync.dma_start(xb[:, T // 2:T], xT[P:C + 1, T // 2:T])

        wva = singles.tile([P, H * 33], BF16)
        nc.gpsimd.dma_start(wva, wv[0:P, :])
        wvb = singles.tile([C + 1 - P, H * 33], BF16)
        nc.gpsimd.dma_start(wvb, wv[P:C + 1, :])
        wpa_sb = singles.tile([96, C], BF16)
        nc.gpsimd.dma_start(wpa_sb, wpa[:, :])
        wpb_sb = singles.tile([97, C], BF16)
        nc.gpsimd.dma_start(wpb_sb, wpb[:, :])
        idn_sb = singles.tile([P, P], BF16)
        nc.gpsimd.dma_start(idn_sb, idn[:, :])

        # PE p-state warmup: ~3.5us of dummy matmuls from a memset tile so
        # the ramp (full clock after 3us continuous busy) completes before
        # the real projections start. PE is idle during the loads anyway.
        warm = singles.tile([P, TCH], BF16)
        nc.gpsimd.memset(warm, 0.0)
        with tc.tile_pool(name="pwarm", bufs=1, space="PSUM") as pwarm:
            wps = pwarm.tile([P, TCH], F32)
            for _ in range(8):
                nc.tensor.matmul(wps, warm[:, 0:P], warm,
                                 start=True, stop=True)

        # ---------------- phase 1: qT, kT, v_aug ----------------
        qT_a = qk_pool.tile([P, T], BF16)       # heads 0..3, d-major
        qT_b = qk_pool.tile([C - P, T], BF16)   # heads 4,5
        kT_a = qk_pool.tile([P, T], BF16)
        kT_b = qk_pool.tile([C - P, T], BF16)
        v_aug = []
        with tc.tile_pool(name="pqk", bufs=2, space="PSUM") as pqk, \
             tc.tile_pool(name="pv", bufs=2, space="PSUM") as pv:
            def v_group(si):
                s0 = si * P
                ps = pv.tile([P, H * 33], F32, name="psv", tag="psv")
                nc.tensor.matmul(ps, xa[:, s0:s0 + P], wva,
                                 start=True, stop=False)
                nc.tensor.matmul(ps, xb[:, s0:s0 + P], wvb,
                                 start=False, stop=True)
                va = vaug_pool.tile([P, H * 33], BF16,
                                    name=f"vaug{si}", tag=f"vaug{si}")
                nc.vector.tensor_copy(va, ps)
                v_aug.append(va)

            # interleave q/k psum groups with v groups so ACT (psq copies)
            # and DVE (vaug copies) both get fed from the start; alternate
            # q/k and put low-t chunks first so QKT(tc0=0) can begin early
            qk_groups = []
            for t0 in range(0, T, 2 * TCH):
                for dlo, dsz, ia in ((0, P, 0), (P, C - P, 1)):
                    qk_groups.append(("q", dlo, dsz,
                                      (qT_a, qT_b)[ia], t0))
                    qk_groups.append(("k", dlo, dsz,
                                      (kT_a, kT_b)[ia], t0))
            vi = 0
            for gi, (proj, dlo, dsz, dst, t0) in enumerate(qk_groups):
                wa, wb = w_sb[proj]
                ps = pqk.tile([P, 2 * TCH], F32, name="psq", tag="psq")
                for tt0 in (t0, t0 + TCH):
                    c0 = tt0 - t0
                    nc.tensor.matmul(
                        ps[0:dsz, c0:c0 + TCH], wa[:, dlo:dlo + dsz],
                        xa[:, tt0:tt0 + TCH], start=True, stop=False)
                    nc.tensor.matmul(
                        ps[0:dsz, c0:c0 + TCH], wb[:, dlo:dlo + dsz],
                        xb[0:C - P, tt0:tt0 + TCH],
                        start=False, stop=True)
                if gi % 2 == 1:
                    nc.scalar.activation(
                        dst[0:dsz, t0:t0 + 2 * TCH], ps[0:dsz, :], Copy,
                        scale=1.0)
                else:
                    nc.vector.tensor_copy(
                        dst[0:dsz, t0:t0 + 2 * TCH], ps[0:dsz, :])
                while vi < NS and vi <= 2 * gi:
                    v_group(vi)
                    vi += 1
            while vi < NS:
                v_group(vi)
                vi += 1

        # ---------------- phase 2: attention ----------------
        def hsrc(h):
            if h < 4:
                return kT_a, qT_a, HS * h
            return kT_b, qT_b, HS * (h - 4)

        exp_i = [0]

        with (
            tc.tile_pool(name="pst", bufs=3, space="PSUM") as pst_pool,
            tc.tile_pool(name="pav", bufs=1, space="PSUM") as pav_pool,
        ):
            pending_tail = [None]
            pending_av = [None]

            def flush_tail():
                if pending_tail[0] is not None:
                    pending_tail[0]()
                    pending_tail[0] = None

            for tci, tc0 in enumerate(range(0, T, TCH)):
                # two av accumulators, each one PSUM bank: cols =
                # (tt%2)*198 + h*33 + [0..32]; col 32 of each head = rowsum
                av = [pav_pool.tile([P, 396], F32, name=f"av{b}", tag=f"av{b}")
                      for b in range(2)]
                pend = {}

                def issue_av(si, av=av, pend=pend):
                    for tt in range(4):
                        b, jj = tt // 2, tt % 2
                        for h in range(H):
                            g, j = h // 2, h % 2
                            ptp = pend[si][g]
                            nc.tensor.matmul(
                                av[b][:, jj * 198 + h * 33:
                                      jj * 198 + (h + 1) * 33],
                                ptp[:, j * TCH + tt * P:
                                    j * TCH + tt * P + P],
                                v_aug[si][:, h * 33:(h + 1) * 33],
                                # start=True marks the WHOLE psum bank
                                # pending-zero, so only the first chain into
                                # each bank may set it; the other chains'
                                # first writes then overwrite pending-zero
                                # bytes (= implicit zero init).
                                start=(si == 0 and jj == 0 and h == 0),
                                stop=(si == NS - 1),
                                skip_group_check=True)

                for si in range(NS):
                    s0 = si * P
                    if si == 2:
                        # emit the previous tc0's tail now: its norms must
                        # precede this tc0's first AV writes (WAR on the av
                        # banks), but deferring it past si=0/1 lets the PE
                        # keep the exp engines fed across the boundary.
                        flush_tail()
                    cur = []
                    for g in range(3):
                        stp = pst_pool.tile([P, 2 * TCH], F32,
                                            name="stp", tag="stp")
                        for j in range(2):
                            h = 2 * g + j
                            kT_t, qT_t, pb = hsrc(h)
                            nc.tensor.matmul(
                                stp[:, j * TCH:(j + 1) * TCH],
                                kT_t[pb:pb + HS, s0:s0 + P],
                                qT_t[pb:pb + HS, tc0:tc0 + TCH],
                                start=True, stop=True, tile_position=(pb, 0))
                        eng, copier = plan[exp_i[0]]
                        exp_i[0] += 1
                        ptp = pt_pool.tile([P, 2 * TCH], BF16,
                                           name="ptp", tag="ptp")
                        if eng == "act":
                            nc.scalar.activation(ptp, stp, Exp, scale=SCALE)
                        else:
                            nc.vector.tensor_scalar(
                                ptp.bitcast(I16), stp, SCH_A * SCALE, SCH_B,
                                op0=MUL, op1=ADD)
                        cur.append(ptp)
                    pend[si] = cur
                    # AV for si-2 issued AFTER this si's QKT groups: the PE
                    # produces stp tiles (exp-engine food) first each round.
                    if si >= 2:
                        issue_av(si - 2)
                issue_av(NS - 2)
                issue_av(NS - 1)

                def tail(av=av, tc0=tc0):
                    # normalize all 4 t-tiles first (frees av banks)
                    # On layout: [h0..h5 (cols 0:192) | ones (col 192)];
                    # group a = cols 0:96, group b = cols 96:193 so the
                    # ones col transposes into otb row 96 (bias row of wpb)
                    ons = []
                    for tt in range(4):
                        b, off = tt // 2, (tt % 2) * 198
                        avr = av[b][:, off:off + 198].rearrange(
                            "p (h e) -> p h e", h=H)
                        rrec = small.tile([P, H], F32, name="rrec", tag="rrec")
                        nc.vector.reciprocal(rrec[:, :, None],
                                             avr[:, :, 32:33])
                        on = on_pool.tile([P, 193], BF16, name="on", tag="on")
                        nc.vector.tensor_tensor(
                            on[:, 0:192].rearrange("p (h e) -> p h e", h=H),
                            avr[:, :, 0:32],
                            rrec[:, :, None].to_broadcast((P, H, 32)),
                            op=MUL)
                        nc.gpsimd.memset(on[:, 192:193], 1.0)
                        ons.append(on)

                    # transpose + project, reusing the freed av banks:
                    # av[0] (bf16 view) holds three 128-col transpose slots,
                    # av[1] holds two 192-col projection-psum slots.
                    av0b = av[0].bitcast(BF16)
                    for tt in range(4):
                        on = ons[tt]
                        ca = 128 * ((2 * tt) % 3)
                        cb = 128 * ((2 * tt + 1) % 3)
                        ga = av0b[:, ca:ca + 128]
                        gb = av0b[:, cb:cb + 128]
                        nc.tensor.transpose(ga[0:96, :], on[:, 0:96], idn_sb)
                        ota = ot_pool.tile([96, P], BF16, name="ota",
                                           tag="ota")
                        nc.scalar.activation(ota, ga[0:96, :], Copy,
                                             scale=1.0)
                        nc.tensor.transpose(gb[0:97, :], on[:, 96:193],
                                            idn_sb)
                        otb = ot_pool.tile([97, P], BF16, name="otb",
                                           tag="otb")
                        nc.vector.tensor_copy(otb, gb[0:97, :])
                        py = av[1][:, (tt % 2) * 192:(tt % 2) * 192 + 192]
                        nc.tensor.matmul(py, ota, wpa_sb,
                                         start=True, stop=False,
                                         skip_group_check=True)
                        nc.tensor.matmul(py, otb, wpb_sb,
                                         start=False, stop=True,
                                         skip_group_check=True)
                        ysb = ysb_pool.tile([P, C], F32, name="ysbt",
                                            tag="ysbt")
                        nc.vector.tensor_copy(ysb, py)
                        nc.sync.dma_start(
                            out[tc0 + tt * P:tc0 + (tt + 1) * P, :], ysb)

                pending_tail[0] = tail
            flush_tail()

    nc.compile()
    return nc


def _get_nc():
    if "nc" not in _CACHE:
        _CACHE["nc"] = build_nc()
    return _CACHE["nc"]


def make_in_maps(x, Wq, Wk, Wv, Wproj, bproj):
    bf = ml_dtypes.bfloat16
    x = np.asarray(x, np.float32)
    pack = lambda w: np.ascontiguousarray(
        np.transpose(np.asarray(w, np.float32), (1, 0, 2)).reshape(C, H * HS)
    ).astype(bf)
    wq, wk = pack(Wq), pack(Wk)

    wv_aug = np.zeros((C + 1, H * 33), np.float32)
    Wv = np.asarray(Wv, np.float32)
    for h in range(H):
        wv_aug[0:C, h * 33:h * 33 + 32] = Wv[h]
        wv_aug[C, h * 33 + 32] = 1.0
    wv_aug = wv_aug.astype(bf)

    Wp = np.asarray(Wproj, np.float32)          # [H*HS, C]
    wpa = np.ascontiguousarray(Wp[0:96]).astype(bf)
    wpb = np.zeros((97, C), np.float32)
    wpb[0:96] = Wp[96:192]
    wpb[96] = np.asarray(bproj, np.float32)
    wpb = wpb.astype(bf)

    idn = np.eye(P, dtype=bf)

    maps = []
    for i in range(B):
        xp = np.ones((C + 1, T), np.float32)
        xp[0:C] = x[i].T
        maps.append({"xT": xp.astype(bf), "wq": wq, "wk": wk,
                     "wv": wv_aug, "wpa": wpa, "wpb": wpb, "idn": idn})
    return maps


def run(inputs, trace=False, **kw):
    nc = _get_nc()
    in_maps = make_in_maps(**inputs)
    res = run_bass_kernel_spmd(nc, in_maps, core_ids=list(range(B)),
                               trace=trace, **kw)
    y = np.stack([np.asarray(res.results[i]["out"], np.float32)
                  for i in range(B)], axis=0)
    return y, res


def kernel(**inputs):
    y, _ = run(inputs, trace=False)
    return y


# revision 56
# speedup vs baseline: 1.0446x; 1.0066x over previous
"""Multi-head self-attention (B=8, T=2048, C=192, H=6, HS=32) on 8 TRN2 cores.

Sharding: data-parallel over batch - core i computes batch element i fully
on-chip (no collectives).

v2 design (cost model: matmul = out-free-cols * pe_cycle; ACT/DVE/Pool =
free-cols * engine cycle):
  - qT/kT [d, t] projections as in v1 (PSUM->SBUF copies on ACT).
  - v_aug [s, 6*33] = x @ Wv_aug with a built-in ones column per head
    (host pads xT with a ones row; Wv_aug carries the ones pattern).
  - S^T [s, t] per head: 1 matmul per (head, s-tile, t-chunk), K=32.
  - P^T = exp(S^T/sqrt(HS)) split across three engines:
      ACT:  exact activation-Exp
      DVE:  Schraudolph bf16-bits exp (int16(A*x+B) bitcast to bf16)
      Pool: same trick from an fp16 staging copy made by ACT/DVE
            (GPSIMD cannot read PSUM)
  - AV in [t, d] orientation: out [t-tile 128, 33] per (t-tile, head,
    s-block) accumulated over 16 s-blocks; col 32 = rowsum via the ones col.
  - normalize via DVE broadcast mul, ones col appended -> On [128, 193] f32
  - PE transpose (fp32, via identity) -> O^T in reused AV psum banks,
    ACT/DVE copy to SBUF -> otnT_a [97, 128] (row 96 = softmax ones ->
    bias via Wp_a row 96 = bproj), otnT_b [96, 128]
  - out projection: 2 matmuls (K=97/96) + DVE copy + DMA per t-tile.
"""

import numpy as np
import ml_dtypes
from contextlib import ExitStack

import concourse.bass as bass
import concourse.tile as tile
from concourse import bacc, mybir
from concourse.bass_utils import run_bass_kernel_spmd

B, T, C = 8, 2048, 192
H, HS = 6, 32
P = 128
TCH = 512            # t-chunk width of one S^T tile pair
NT = T // TCH        # 4
NS = T // P          # 16 s-tiles / t-tiles
SCALE = 1.0 / float(np.sqrt(HS))
BF16 = mybir.dt.bfloat16
F16 = mybir.dt.float16
F32 = mybir.dt.float32
I16 = mybir.dt.int16
Exp = mybir.ActivationFunctionType.Exp
Copy = mybir.ActivationFunctionType.Copy
MUL = mybir.AluOpType.mult
ADD = mybir.AluOpType.add

# Schraudolph constants for bf16-bits exp: bf16_bits = int16(A*x + B)
SCH_A = 128.0 / np.log(2.0)
SCH_B = 127.0 * 128.0 - 7.5 + 0.5

# exp-engine split (192 tiles of [128, 1024]): ACT direct / DVE direct.
# Pool exp is a net loss (GPSIMD can't read PSUM and the staging copy
# costs the copier engine as much as doing the exp directly).
N_ACT, N_DVE, N_POOL = 105, 87, 0
N_POOLCP_ACT = 0      # of the pool tiles, how many staging copies ACT makes
N_EXP = 192

_CACHE = {}


def _exp_plan():
    """Weighted round-robin: list of (engine, copier) for the exp tiles."""
    plan = []
    acc = {"act": 0.0, "dve": 0.0, "pool": 0.0}
    tot = float(N_EXP)
    w = {"act": N_ACT / tot, "dve": N_DVE / tot, "pool": N_POOL / tot}
    n = {"act": 0, "dve": 0, "pool": 0}
    cap = {"act": N_ACT, "dve": N_DVE, "pool": N_POOL}
    cp_acc, cp_n = 0.0, 0
    for _ in range(N_EXP):
        for k in acc:
            acc[k] += w[k]
        pick = max((k for k in acc if n[k] < cap[k]), key=lambda k: acc[k])
        acc[pick] -= 1.0
        n[pick] += 1
        copier = None
        if pick == "pool":
            cp_acc += N_POOLCP_ACT / float(N_POOL)
            if cp_acc >= 1.0 and cp_n < N_POOLCP_ACT:
                cp_acc -= 1.0
                cp_n += 1
                copier = "act"
            else:
                copier = "dve"
        plan.append((pick, copier))
    return plan


def build_nc():
    nc = bacc.Bacc()
    xT = nc.declare_dram_parameter("xT", [C + 1, T], BF16, isOutput=False)
    wq = nc.declare_dram_parameter("wq", [C, H * HS], BF16, isOutput=False)
    wk = nc.declare_dram_parameter("wk", [C, H * HS], BF16, isOutput=False)
    wv = nc.declare_dram_parameter("wv", [C + 1, H * 33], BF16, isOutput=False)
    wpa = nc.declare_dram_parameter("wpa", [96, C], BF16, isOutput=False)
    wpb = nc.declare_dram_parameter("wpb", [97, C], BF16, isOutput=False)
    idn = nc.declare_dram_parameter("idn", [P, P], BF16, isOutput=False)
    out = nc.declare_dram_parameter("out", [T, C], F32, isOutput=True)

    plan = _exp_plan()

    with tile.TileContext(nc) as tc, ExitStack() as ctx:
        singles = ctx.enter_context(tc.tile_pool(name="singles", bufs=1))
        qk_pool = ctx.enter_context(tc.tile_pool(name="qk", bufs=1))
        vaug_pool = ctx.enter_context(tc.tile_pool(name="vaug", bufs=1))
        pt_pool = ctx.enter_context(tc.tile_pool(name="ptp", bufs=18))
        on_pool = ctx.enter_context(tc.tile_pool(name="onp", bufs=4))
        ot_pool = ctx.enter_context(tc.tile_pool(name="otp", bufs=2))
        small = ctx.enter_context(tc.tile_pool(name="small", bufs=4))
        ysb_pool = ctx.enter_context(tc.tile_pool(name="ysb", bufs=3))

        # ---------------- load inputs ----------------
        # Spread the loads over four engine DMA queues so the first q/k
        # projection group's inputs (wq a/b + x chunk 0) are all in flight
        # immediately instead of serializing on the SP queue.
        w_sb = {}
        wqa_t = singles.tile([P, H * HS], BF16, name="wqa")
        nc.sync.dma_start(wqa_t, wq[0:P, :])
        wqb_t = singles.tile([C - P, H * HS], BF16, name="wqb")
        nc.scalar.dma_start(wqb_t, wq[P:C, :])
        wka_t = singles.tile([P, H * HS], BF16, name="wka")
        nc.gpsimd.dma_start(wka_t, wk[0:P, :])
        wkb_t = singles.tile([C - P, H * HS], BF16, name="wkb")
        nc.gpsimd.dma_start(wkb_t, wk[P:C, :])
        w_sb["q"] = (wqa_t, wqb_t)
        w_sb["k"] = (wka_t, wkb_t)

        xa = singles.tile([P, T], BF16)
        xb = singles.tile([C + 1 - P, T], BF16)          # 65 rows (ones last)
        nc.sync.dma_start(xa[:, 0:T // 2], xT[0:P, 0:T // 2])
        nc.scalar.dma_start(xb[:, 0:T // 2], xT[P:C + 1, 0:T // 2])
        nc.sync.dma_start(xa[:, T // 2:T], xT[0:P, T // 2:T])
        nc.sync.dma_start(xb[:, T // 2:T], xT[P:C + 1, T // 2:T])

        wva = singles.tile([P, H * 33], BF16)
        nc.gpsimd.dma_start(wva, wv[0:P, :])
        wvb = singles.tile([C + 1 - P, H * 33], BF16)
        nc.gpsimd.dma_start(wvb, wv[P:C + 1, :])
        wpa_sb = singles.tile([96, C], BF16)
        nc.gpsimd.dma_start(wpa_sb, wpa[:, :])
        wpb_sb = singles.tile([97, C], BF16)
        nc.gpsimd.dma_start(wpb_sb, wpb[:, :])
        idn_sb = singles.tile([P, P], BF16)
        nc.gpsimd.dma_start(idn_sb, idn[:, :])

        # PE p-state warmup: ~3.5us of dummy matmuls from a memset tile so
        # the ramp (full clock after 3us continuous busy) completes before
        # the real projections start. PE is idle during the loads anyway.
        warm = singles.tile([P, TCH], BF16)
        nc.gpsimd.memset(warm, 0.0)
        with tc.tile_pool(name="pwarm", bufs=1, space="PSUM") as pwarm:
            wps = pwarm.tile([P, TCH], F32)
            for _ in range(8):
                nc.tensor.matmul(wps, warm[:, 0:P], warm,
                                 start=True, stop=True)

        # ---------------- phase 1: qT, kT, v_aug ----------------
        qT_a = qk_pool.tile([P, T], BF16)       # heads 0..3, d-major
        qT_b = qk_pool.tile([C - P, T], BF16)   # heads 4,5
        kT_a = qk_pool.tile([P, T], BF16)
        kT_b = qk_pool.tile([C - P, T], BF16)
        v_aug = []
        with tc.tile_pool(name="pqk", bufs=3, space="PSUM") as pqk, \
             tc.tile_pool(name="pv", bufs=2, space="PSUM") as pv:
            def v_group(si):
                s0 = si * P
                ps = pv.tile([P, H * 33], F32, name="psv", tag="psv")
                nc.tensor.matmul(ps, xa[:, s0:s0 + P], wva,
                                 start=True, stop=False)
                nc.tensor.matmul(ps, xb[:, s0:s0 + P], wvb,
                                 start=False, stop=True)
                va = vaug_pool.tile([P, H * 33], BF16,
                                    name=f"vaug{si}", tag=f"vaug{si}")
                nc.vector.tensor_copy(va, ps)
                v_aug.append(va)

            # interleave q/k psum groups with v groups so ACT (psq copies)
            # and DVE (vaug copies) both get fed from the start; alternate
            # q/k and put low-t chunks first so QKT(tc0=0) can begin early
            qk_groups = []
            for t0 in range(0, T, 2 * TCH):
                for dlo, dsz, ia in ((0, P, 0), (P, C - P, 1)):
                    qk_groups.append(("q", dlo, dsz,
                                      (qT_a, qT_b)[ia], t0))
                    qk_groups.append(("k", dlo, dsz,
                                      (kT_a, kT_b)[ia], t0))
            vi = 0
            for gi, (proj, dlo, dsz, dst, t0) in enumerate(qk_groups):
                wa, wb = w_sb[proj]
                ps = pqk.tile([P, 2 * TCH], F32, name="psq", tag="psq")
                for tt0 in (t0, t0 + TCH):
                    c0 = tt0 - t0
                    nc.tensor.matmul(
                        ps[0:dsz, c0:c0 + TCH], wa[:, dlo:dlo + dsz],
                        xa[:, tt0:tt0 + TCH], start=True, stop=False)
                    nc.tensor.matmul(
                        ps[0:dsz, c0:c0 + TCH], wb[:, dlo:dlo + dsz],
                        xb[0:C - P, tt0:tt0 + TCH],
                        start=False, stop=True)
                nc.scalar.activation(
                    dst[0:dsz, t0:t0 + 2 * TCH], ps[0:dsz, :], Copy,
                    scale=1.0)
                while vi < NS and vi <= 2 * gi:
                    v_group(vi)
                    vi += 1
            while vi < NS:
                v_group(vi)
                vi += 1

        # ---------------- phase 2: attention ----------------
        def hsrc(h):
            if h < 4:
                return kT_a, qT_a, HS * h
            return kT_b, qT_b, HS * (h - 4)

        exp_i = [0]

        with (
            tc.tile_pool(name="pst", bufs=3, space="PSUM") as pst_pool,
            tc.tile_pool(name="pav", bufs=1, space="PSUM") as pav_pool,
        ):
            pending_tail = [None]
            pending_av = [None]

            def flush_tail():
                if pending_tail[0] is not None:
                    pending_tail[0]()
                    pending_tail[0] = None

            for tci, tc0 in enumerate(range(0, T, TCH)):
                # two av accumulators, each one PSUM bank: cols =
                # (tt%2)*198 + h*33 + [0..32]; col 32 of each head = rowsum
                av = [pav_pool.tile([P, 396], F32, name=f"av{b}", tag=f"av{b}")
                      for b in range(2)]
                pend = {}

                def issue_av(si, av=av, pend=pend):
                    for tt in range(4):
                        b, jj = tt // 2, tt % 2
                        for h in range(H):
                            g, j = h // 2, h % 2
                            ptp = pend[si][g]
                            nc.tensor.matmul(
                                av[b][:, jj * 198 + h * 33:
                                      jj * 198 + (h + 1) * 33],
                                ptp[:, j * TCH + tt * P:
                                    j * TCH + tt * P + P],
                                v_aug[si][:, h * 33:(h + 1) * 33],
                                # start=True marks the WHOLE psum bank
                                # pending-zero, so only the first chain into
                                # each bank may set it; the other chains'
                                # first writes then overwrite pending-zero
                                # bytes (= implicit zero init).
                                start=(si == 0 and jj == 0 and h == 0),
                                stop=(si == NS - 1),
                                skip_group_check=True)

                for si in range(NS):
                    s0 = si * P
                    if si == 2:
                        # emit the previous tc0's tail now: its norms must
                        # precede this tc0's first AV writes (WAR on the av
                        # banks), but deferring it past si=0/1 lets the PE
                        # keep the exp engines fed across the boundary.
                        flush_tail()
                    cur = []
                    for g in range(3):
                        stp = pst_pool.tile([P, 2 * TCH], F32,
                                            name="stp", tag="stp")
                        for j in range(2):
                            h = 2 * g + j
                            kT_t, qT_t, pb = hsrc(h)
                            nc.tensor.matmul(
                                stp[:, j * TCH:(j + 1) * TCH],
                                kT_t[pb:pb + HS, s0:s0 + P],
                                qT_t[pb:pb + HS, tc0:tc0 + TCH],
                                start=True, stop=True, tile_position=(pb, 0))
                        eng, copier = plan[exp_i[0]]
                        exp_i[0] += 1
                        ptp = pt_pool.tile([P, 2 * TCH], BF16,
                                           name="ptp", tag="ptp")
                        if eng == "act":
                            nc.scalar.activation(ptp, stp, Exp, scale=SCALE)
                        else:
                            nc.vector.tensor_scalar(
                                ptp.bitcast(I16), stp, SCH_A * SCALE, SCH_B,
                                op0=MUL, op1=ADD)
                        cur.append(ptp)
                    pend[si] = cur
                    # AV for si-2 issued AFTER this si's QKT groups: the PE
                    # produces stp tiles (exp-engine food) first each round.
                    if si >= 2:
                        issue_av(si - 2)
                issue_av(NS - 2)
                issue_av(NS - 1)

                def tail(av=av, tc0=tc0):
                    # normalize all 4 t-tiles first (frees av banks)
                    # On layout: [h0..h5 (cols 0:192) | ones (col 192)];
                    # group a = cols 0:96, group b = cols 96:193 so the
                    # ones col transposes into otb row 96 (bias row of wpb)
                    ons = []
                    for tt in range(4):
                        b, off = tt // 2, (tt % 2) * 198
                        avr = av[b][:, off:off + 198].rearrange(
                            "p (h e) -> p h e", h=H)
                        rrec = small.tile([P, H], F32, name="rrec", tag="rrec")
                        nc.vector.reciprocal(rrec[:, :, None],
                                             avr[:, :, 32:33])
                        on = on_pool.tile([P, 193], BF16, name="on", tag="on")
                        nc.vector.tensor_tensor(
                            on[:, 0:192].rearrange("p (h e) -> p h e", h=H),
                            avr[:, :, 0:32],
                            rrec[:, :, None].to_broadcast((P, H, 32)),
                            op=MUL)
                        nc.gpsimd.memset(on[:, 192:193], 1.0)
                        ons.append(on)

                    # transpose + project, reusing the freed av banks:
                    # av[0] (bf16 view) holds three 128-col transpose slots,
                    # av[1] holds two 192-col projection-psum slots.
                    av0b = av[0].bitcast(BF16)
                    for tt in range(4):
                        on = ons[tt]
                        ca = 128 * ((2 * tt) % 3)
                        cb = 128 * ((2 * tt + 1) % 3)
                        ga = av0b[:, ca:ca + 128]
                        gb = av0b[:, cb:cb + 128]
                        nc.tensor.transpose(ga[0:96, :], on[:, 0:96], idn_sb)
                        ota = ot_pool.tile([96, P], BF16, name="ota",
                                           tag="ota")
                        nc.scalar.activation(ota, ga[0:96, :], Copy,
                                             scale=1.0)
                        nc.tensor.transpose(gb[0:97, :], on[:, 96:193],
                                            idn_sb)
                        otb = ot_pool.tile([97, P], BF16, name="otb",
                                           tag="otb")
                        nc.vector.tensor_copy(otb, gb[0:97, :])
                        py = av[1][:, (tt % 2) * 192:(tt % 2) * 192 + 192]
                        nc.tensor.matmul(py, ota, wpa_sb,
                                         start=True, stop=False,
                                         skip_group_check=True)
                        nc.tensor.matmul(py, otb, wpb_sb,
                                         start=False, stop=True,
                                         skip_group_check=True)
                        ysb = ysb_pool.tile([P, C], F32, name="ysbt",
                                            tag="ysbt")
                        nc.vector.tensor_copy(ysb, py)
                        nc.sync.dma_start(
                            out[tc0 + tt * P:tc0 + (tt + 1) * P, :], ysb)

                pending_tail[0] = tail
            flush_tail()

    nc.compile()
    return nc


def _get_nc():
    if "nc" not in _CACHE:
        _CACHE["nc"] = build_nc()
    return _CACHE["nc"]


def make_in_maps(x, Wq, Wk, Wv, Wproj, bproj):
    bf = ml_dtypes.bfloat16
    x = np.asarray(x, np.float32)
    pack = lambda w: np.ascontiguousarray(
        np.transpose(np.asarray(w, np.float32), (1, 0, 2)).reshape(C, H * HS)
    ).astype(bf)
    wq, wk = pack(Wq), pack(Wk)

    wv_aug = np.zeros((C + 1, H * 33), np.float32)
    Wv = np.asarray(Wv, np.float32)
    for h in range(H):
        wv_aug[0:C, h * 33:h * 33 + 32] = Wv[h]
        wv_aug[C, h * 33 + 32] = 1.0
    wv_aug = wv_aug.astype(bf)

    Wp = np.asarray(Wproj, np.float32)          # [H*HS, C]
    wpa = np.ascontiguousarray(Wp[0:96]).astype(bf)
    wpb = np.zeros((97, C), np.float32)
    wpb[0:96] = Wp[96:192]
    wpb[96] = np.asarray(bproj, np.float32)
    wpb = wpb.astype(bf)

    idn = np.eye(P, dtype=bf)

    maps = []
    for i in range(B):
        xp = np.ones((C + 1, T), np.float32)
        xp[0:C] = x[i].T
        maps.append({"xT": xp.astype(bf), "wq": wq, "wk": wk,
                     "wv": wv_aug, "wpa": wpa, "wpb": wpb, "idn": idn})
    return maps


def run(inputs, trace=False, **kw):
    nc = _get_nc()
    in_maps = make_in_maps(**inputs)
    res = run_bass_kernel_spmd(nc, in_maps, core_ids=list(range(B)),
                               trace=trace, **kw)
    y = np.stack([np.asarray(res.results[i]["out"], np.float32)
                  for i in range(B)], axis=0)
    return y, res


def kernel(**inputs):
    y, _ = run(inputs, trace=False)
    return y
